# revision 31
# baseline (speedup 1.0000x reference)
"""GAT (2-layer) + global mean pool + MLP + log_softmax on 8 Trainium2 cores.

v4 deltas over v3 (trace-driven):
  - S/ST one-hot tiles shipped as fp8 (halves the dominant DMA stream that
    was ~24ms of aggregate engine busy), converted to f16 on-chip with the
    convert split across the scalar (S) and vector (ST) engines.
  - Edge weight x applied to the gathered rhs rows ([P,QT,65], half the
    elements) instead of folding into the one-hot ([P,QT,128]).
  - AllGather outputs allocated in Shared DRAM scratchpad (fast HBM-HBM
    collective path; Local outputs warn + take a slow path).
  - Pool matmuls accumulate across a chunk in PSUM (one SBUF add per chunk).
  - Softmax denominator is strictly positive (self-loop), so the 1e-30
    clamp is dropped and normalization runs on the scalar engine
    (activation with per-partition scale=1/den).

Strategy (dst-sharded message passing, v3):
  - Nodes partitioned contiguously across 8 cores (12500 each). Each core
    aggregates messages for its own destination nodes.
  - Node tables are f16 rows [h@W (64) | 1 | exp(as) | exp(0.2*as)] padded
    to 128 elems (256B), built shard-wise, replicated via AllGather.  A
    per-core side table eds holds [exp(ad) | exp(0.2*ad)] per local node.
    Per-edge attention weight
      m = exp(leaky_relu(as+ad, 0.2)) == max(e^as*e^ad, e^.2as*e^.2ad)
    exactly, so the scalar engine only ever runs Exp on [128,1] columns
    per block - no per-edge activations, no activation-table thrash.
  - Gathers use gpsimd.dma_gather (one ~1us Q7 call amortized over
    thousands of 256B-row descriptors).  Its int16 index limit forces
    splitting the 100352-row table into 4 source "quarters"; edge cells
    are grouped (block, quarter) with per-(block,quarter) column padding.
    Self-loops are excluded from the gather entirely and added per block
    via an identity matmul on locally-loaded rows.
  - Per (block,quarter): one 3D-broadcast is_equal builds the one-hot
    tiles, one 3D mult scales rhs tiles by m (the table's constant-1
    column turns into the softmax denominator).  The PE accumulates
    acc[dst,0:65] = sum_e onehot^T @ (m*[h|1]) over edge tiles in PSUM.
  - Graph pooling = one-hot matmul per block summed in SBUF, AllReduce,
    and the tiny MLP + log_softmax run redundantly on every core.
"""

import sys

sys.path.insert(0, "/opt/trn_rl_repo")

import ml_dtypes
import numpy as np

F8 = ml_dtypes.float8_e4m3fn

P = 128
Q = 4          # source quarters (table split so indices fit int16)
ELEM = 128     # table row width in f16 elems (256B, dma_gather granularity)


def _wrap16(flat):
    """dma_gather index layout: index j at [j%16, j//16], replicated x8."""
    w = flat.reshape(-1, 16).T
    return np.tile(w, (8, 1)).astype(np.int16)


def _build_host_data(x, edge_index, batch, W1, a_src1, a_dst1, W2, a_src2, a_dst2,
                     ncores):
    """Pure-integer/graph preprocessing + augmented weights (host side)."""
    N, F_IN = x.shape
    HID = W1.shape[1]
    assert N % ncores == 0
    npc = N // ncores
    nblk = (npc + P - 1) // P
    npc_pad = nblk * P
    V = ncores * npc_pad
    assert V % Q == 0
    VQ = V // Q
    assert VQ <= 32768

    # self-loops are handled separately (identity matmul per block)
    src = edge_index[0].astype(np.int64)
    dst = edge_index[1].astype(np.int64)
    keep = src != dst
    src, dst = src[keep], dst[keep]

    src_pad = (src // npc) * npc_pad + (src % npc)
    core_of = dst // npc
    dst_loc = dst - core_of * npc
    blk_of = dst_loc // P
    dst_rel = dst_loc % P
    quart = src_pad // VQ

    counts = np.zeros((ncores, nblk, Q), np.int64)
    np.add.at(counts, (core_of, blk_of, quart), 1)
    tqb = int(max(1, np.ceil(counts.max() / P)))

    # chunk size: largest divisor of nblk with staging tiles <= ~9KB/partition
    CH = 1
    for ch in range(1, nblk + 1):
        if nblk % ch == 0 and ch * tqb * ELEM * 2 <= 10 * 1024:
            CH = ch
    nchunks = nblk // CH

    ncol_blk = Q * tqb
    ncols = nblk * ncol_blk

    idx16_all = np.zeros((ncores, P, ncols * 8), np.int16)
    # one-hot tiles (S) and their per-tile transposes (ST), uploaded as fp8:
    # S[e, col*128+c] = 1 iff edge at (e, col) targets dst_rel c
    S_all = np.zeros((ncores, P, ncols * P), F8)
    ST_all = np.zeros((ncores, P, ncols * P), F8)

    order = np.lexsort((dst_loc, quart, blk_of, core_of))
    co, bo, qo, ro, sp, dl = (core_of[order], blk_of[order], quart[order],
                              dst_rel[order], src_pad[order], dst_loc[order])
    key = (co * nblk + bo) * Q + qo
    start = np.searchsorted(key, np.arange(ncores * nblk * Q), side="left")
    pos = np.arange(len(key)) - start[key]
    t_of = pos // P
    p_of = pos % P
    chunk = bo // CH
    b_loc = bo % CH

    # block-major column id (block's Q*tqb columns contiguous)
    col = (bo * Q + qo) * tqb + t_of
    ro32 = ro.astype(np.int64)
    S_all[co, p_of, col * P + ro32] = 1.0
    ST_all[co, ro32, col * P + p_of] = 1.0

    # table-gather indices: call (chunk, q), flat j = (b_loc*tqb+t)*128 + p
    jf = (b_loc * tqb + t_of) * P + p_of
    call_col0 = chunk * (Q * CH * tqb * 8) + qo * (CH * tqb * 8)
    r16 = (jf % 16)
    c16 = jf // 16
    val = (sp - qo * VQ).astype(np.int16)
    for rep in range(8):
        idx16_all[co, r16 + 16 * rep, call_col0 + c16] = val

    gid_all = np.full((ncores, P, nblk), -1.0, np.float32)
    for c in range(ncores):
        ids = np.arange(npc) + c * npc
        g = batch[ids].astype(np.float32)
        gg = np.full(npc_pad, -1.0, np.float32)
        gg[:npc] = g
        gid_all[c] = gg.reshape(nblk, P).T

    xT_all = np.zeros((ncores, F_IN, npc_pad), np.float16)
    for c in range(ncores):
        xT_all[c, :, :npc] = x[c * npc:(c + 1) * npc].T.astype(np.float16)

    W1aug = np.concatenate([W1, (W1 @ a_src1)[:, None], (W1 @ a_dst1)[:, None]],
                           axis=1).astype(np.float16)
    W2aug = np.concatenate([W2, (W2 @ a_src2)[:, None], (W2 @ a_dst2)[:, None]],
                           axis=1).astype(np.float16)

    # interleave per block: [S_b | ST_b] so the device fetches both in one DMA
    ncol_blk2 = Q * tqb
    SST_all = np.concatenate(
        [S_all.reshape(ncores, P, nblk, ncol_blk2 * P),
         ST_all.reshape(ncores, P, nblk, ncol_blk2 * P)],
        axis=3).reshape(ncores, P, nblk * 2 * ncol_blk2 * P)

    return dict(npc=npc, nblk=nblk, npc_pad=npc_pad, tqb=tqb, ncols=ncols,
                CH=CH, nchunks=nchunks, VQ=VQ,
                SST_all=SST_all, idx16_all=idx16_all,
                gid_all=gid_all, xT_all=xT_all, W1aug=W1aug, W2aug=W2aug)


def _build_program(ncores, nblk, tqb, CH, F_IN, HID, G, C):
    import concourse.bass as bass
    import concourse.bacc as bacc
    import concourse.tile as tile
    from concourse import mybir

    TW = HID + 3          # used row prefix: [h | 1 | es | es2]
    HM = HID + 1          # matmul rhs width: [m*h | m]
    npc_pad = nblk * P
    V = ncores * npc_pad
    VQ = V // Q
    nchunks = nblk // CH
    ncols = nblk * Q * tqb
    NT = CH * tqb * P     # indices per table-gather call

    nc = bacc.Bacc("TRN2", target_bir_lowering=False, debug=False,
                   num_devices=ncores, num_swdge_queues=4)
    f32, f16 = mybir.dt.float32, mybir.dt.float16
    f8 = mybir.dt.float8e4
    i16 = mybir.dt.int16
    Alu = mybir.AluOpType
    Act = mybir.ActivationFunctionType

    ein = lambda n, s, d: nc.dram_tensor(n, s, d, kind="ExternalInput")
    xT_d = ein("xT", [F_IN, npc_pad], f16)
    idx16_d = ein("idx16", [P, ncols * 8], i16)
    # S and ST one-hot tiles interleaved per block: [S_b | ST_b], fp8
    sst_d = ein("sst", [P, ncols * P * 2], f8)
    gid_d = ein("gid", [P, nblk], f32)
    w1_d = ein("w1aug", [F_IN, HID + 2], f16)
    w2_d = ein("w2aug", [HID, HID + 2], f16)
    b1_d = ein("b1rep", [P, HID], f32)
    b2_d = ein("b2rep", [P, HID], f32)
    lw_d = ein("lin_w", [HID, HID // 2], f32)
    lb_d = ein("lin_b", [HID // 2, 1], f32)
    cw_d = ein("cls_w", [HID // 2, C], f32)
    cb_d = ein("cls_b", [C, 1], f32)
    io32_d = ein("iota32", [P, P], f32)
    id16_d = ein("ident16", [P, P], f16)
    id32_d = ein("ident32", [P, P], f32)
    one16_d = ein("ones16", [P, 1], f16)
    out_d = nc.dram_tensor("out", [G, C], f32, kind="ExternalOutput")

    with tile.TileContext(nc) as tc:
        with (
            tc.tile_pool(name="cst", bufs=1) as cst,
            tc.tile_pool(name="sb", bufs=6) as sb,
            tc.tile_pool(name="gq", bufs=12) as gq,
            tc.tile_pool(name="psA", bufs=2, space="PSUM") as psA,
            tc.tile_pool(name="ps", bufs=1, space="PSUM") as ps,
            tc.tile_pool(name="psacc", bufs=2, space="PSUM") as psacc,
            tc.tile_pool(name="dram", bufs=1, space="DRAM") as dram,
        ):
            # ---- constants ----
            gid_t = cst.tile([P, nblk], f32)
            nc.sync.dma_start(gid_t[:], gid_d[:, :])
            w1_t = cst.tile([F_IN, HID + 2], f16)
            nc.sync.dma_start(w1_t[:], w1_d[:, :])
            w2_t = cst.tile([HID, HID + 2], f16)
            nc.sync.dma_start(w2_t[:], w2_d[:, :])
            b1_t = cst.tile([P, HID], f32)
            nc.sync.dma_start(b1_t[:], b1_d[:, :])
            b2_t = cst.tile([P, HID], f32)
            nc.sync.dma_start(b2_t[:], b2_d[:, :])
            io32_t = cst.tile([P, P], f32)
            nc.sync.dma_start(io32_t[:], io32_d[:, :])
            id16_t = cst.tile([P, P], f16)
            nc.sync.dma_start(id16_t[:], id16_d[:, :])
            id32_t = cst.tile([P, P], f32)
            nc.sync.dma_start(id32_t[:], id32_d[:, :])
            one16_t = cst.tile([P, 1], f16)
            nc.sync.dma_start(one16_t[:], one16_d[:, :])

            # DRAM: local table slices + replicated tables + dst exp pairs.
            # Tables live in the Shared scratchpad so the AllGather takes the
            # fast HBM-HBM path (Local outputs warn + bounce).
            slice1 = dram.tile([npc_pad, ELEM], f16)
            slice2 = dram.tile([npc_pad, ELEM], f16)
            table1 = nc.dram_tensor("table1_sh", [V, ELEM], f16,
                                    kind="Internal", addr_space="Shared")
            table2 = nc.dram_tensor("table2_sh", [V, ELEM], f16,
                                    kind="Internal", addr_space="Shared")
            # per-block row of exp(0.8*ad) per local node
            eds1 = dram.tile([nblk, P], f16)
            eds2 = dram.tile([nblk, P], f16)
            pool_in = dram.tile([P, HM], f32)
            pool_out = dram.tile([P, HM], f32)

            # pooled accumulator (SBUF, across all blocks of layer 2)
            pooled = cst.tile([P, HM], f32)
            nc.vector.memset(pooled[:], 0.0)

            def node_transform(src_ps, dst_sb, ed_sb):
                """src_ps[:, 0:HID+2] = [h | as | ad] (f32, PSUM) ->
                dst_sb[:,0:TW] = [h | 1 | e^as | e^.2as] (f16),
                ed_sb[:,0:1]   = e^(0.8*ad) (f16)."""
                nc.scalar.activation(dst_sb[:, 0:HID], src_ps[:, 0:HID],
                                     Act.Identity)
                nc.vector.tensor_copy(dst_sb[:, HID:HID + 1], one16_t[:])
                nc.scalar.activation(dst_sb[:, HID + 1:HID + 2],
                                     src_ps[:, HID:HID + 1], Act.Exp)
                nc.scalar.activation(dst_sb[:, HID + 2:HID + 3],
                                     src_ps[:, HID:HID + 1], Act.Exp, scale=0.2)
                nc.scalar.activation(ed_sb[:, 0:1],
                                     src_ps[:, HID + 1:HID + 2], Act.Exp,
                                     scale=0.8)

            # ---- phase 0: table1 rows from x@W1aug (python-unrolled) ----
            for b in range(nblk):
                xb = sb.tile([F_IN, P], f16, tag="xb")
                nc.sync.dma_start(xb[:], xT_d[:, bass.ds(b * P, P)])
                t1_ps = psA.tile([P, HID + 2], f32, tag="p0ps")
                nc.tensor.matmul(t1_ps[:], lhsT=xb[:], rhs=w1_t[:],
                                 start=True, stop=True)
                t1_sb = sb.tile([P, TW], f16, tag="p0sb")
                ed_sb = sb.tile([P, 1], f16, tag="p0ed")
                node_transform(t1_ps, t1_sb, ed_sb)
                nc.scalar.dma_start(slice1[bass.ds(b * P, P), 0:TW], t1_sb[:])
                nc.scalar.dma_start(
                    eds1[b:b + 1, :].rearrange("a (b c) -> (a b) c", c=1),
                    ed_sb[:])

            nc.gpsimd.collective_compute(
                "AllGather", Alu.bypass,
                replica_groups=[list(range(ncores))],
                ins=[slice1.opt()], outs=[table1[:, :]],
            )

            def gat_layer(table_ap, eds_ap, slice_ap, out_slice_ap, out_eds_ap,
                          is_last):
                def block_prep(c, k, g_q):
                    """Compute acc (PSUM) for block k of chunk c.

                    Attention weight per edge: m' = max(es_e*ed8_c, es2_e)
                    with ed8 = e^(0.8*ad).  The exact weight is
                    e^(0.2*ad_c) * m', but that factor is constant per dst
                    row and cancels in the softmax numerator/denominator."""
                    # self-loop row for this block (local, no gather)
                    g_self = sb.tile([P, TW], f16, tag="gself")
                    nc.sync.dma_start(
                        g_self[:],
                        slice_ap[bass.ds(c * (CH * P) + k * P, P), 0:TW])
                    ed8col = sb.tile([P, 1], f16, tag="edself")
                    nc.sync.dma_start(
                        ed8col[:],
                        eds_ap[c * CH + k:c * CH + k + 1, :]
                        .rearrange("a (b c) -> (a b) c", c=1))
                    u_s = sb.tile([P, 1], f16, tag="uws")
                    nc.vector.tensor_tensor(out=u_s[:],
                                            in0=g_self[:, HID + 1:HID + 2],
                                            in1=ed8col[:], op=Alu.mult)
                    m_s = sb.tile([P, 1], f16, tag="ms")
                    nc.vector.tensor_tensor(out=m_s[:], in0=u_s[:],
                                            in1=g_self[:, HID + 2:HID + 3],
                                            op=Alu.max)
                    rhs_s = sb.tile([P, HM], f16, tag="rhss")
                    nc.vector.tensor_tensor(out=rhs_s[:],
                                            in0=g_self[:, 0:HM],
                                            in1=m_s[:].to_broadcast([P, HM]),
                                            op=Alu.mult)

                    acc = psacc.tile([P, HM], f32, tag="acc")
                    nc.tensor.matmul(acc[:], lhsT=id16_t[:], rhs=rhs_s[:],
                                     start=True, stop=False)

                    QT = Q * tqb
                    # stream this block's one-hot tiles (host-precomputed fp8,
                    # S and ST in one DMA); the PE consumes fp8 lhsT directly
                    sst_up = sb.tile([P, 2 * QT * P], f8, tag="SST")
                    nc.sync.dma_start(
                        sst_up[:],
                        sst_d[:, bass.ds((c * CH + k) * 2 * QT * P,
                                         2 * QT * P)])
                    S8 = sst_up[:, 0:QT * P]
                    ST8 = sst_up[:, QT * P:2 * QT * P]

                    # per-edge ed8 of the dst via transposed one-hot matmuls
                    edps = ps.tile([P, QT], f32, tag="edR")
                    for j in range(QT):
                        nc.tensor.matmul(edps[:, j:j + 1],
                                         lhsT=ST8[:, bass.ds(j * P, P)],
                                         rhs=ed8col[:], start=True, stop=True)

                    # per-edge [es|es2] pairs for all 4 quarters into one tile
                    esb = sb.tile([P, QT * 2], f16, tag="uw")
                    for q in range(Q):
                        g3 = g_q[q][:].rearrange("p (n e) -> p n e", e=ELEM)[
                            :, k * tqb:(k + 1) * tqb, :]
                        nc.vector.tensor_copy(
                            esb[:, bass.ds(q * tqb * 2, tqb * 2)]
                                .rearrange("p (t c) -> p t c", t=tqb),
                            g3[:, :, HID + 1:HID + 3])
                    es3 = esb[:].rearrange("p (t c) -> p t c", t=QT)
                    # x_e = max(es_e * ed8_dst(e), es2_e)  [P, QT]
                    x = sb.tile([P, QT], f16, tag="m")
                    nc.vector.tensor_tensor(
                        out=x[:].unsqueeze(2), in0=es3[:, :, 0:1],
                        in1=edps[:].unsqueeze(2), op=Alu.mult)
                    nc.vector.tensor_tensor(
                        out=x[:].unsqueeze(2), in0=x[:].unsqueeze(2),
                        in1=es3[:, :, 1:2], op=Alu.max)
                    # scale the gathered rhs rows by x ([h|1] -> x*[h|1], so
                    # acc col 64 accumulates the softmax denominator); cheaper
                    # than folding x into the 128-wide one-hot
                    rhs_s = sb.tile([P, QT * HM], f16, tag="rhsS")
                    for q in range(Q):
                        g3 = g_q[q][:].rearrange("p (n e) -> p n e", e=ELEM)[
                            :, k * tqb:(k + 1) * tqb, :]
                        nc.vector.tensor_tensor(
                            out=rhs_s[:, bass.ds(q * tqb * HM, tqb * HM)]
                                .rearrange("p (t c) -> p t c", t=tqb),
                            in0=g3[:, :, 0:HM],
                            in1=x[:, bass.ds(q * tqb, tqb)].unsqueeze(2)
                                .to_broadcast([P, tqb, HM]),
                            op=Alu.mult)
                    for j in range(QT):
                        nc.tensor.matmul(
                            acc[:],
                            lhsT=S8[:, bass.ds(j * P, P)],
                            rhs=rhs_s[:, bass.ds(j * HM, HM)],
                            start=False,
                            stop=(j == QT - 1))
                    return acc

                def block_epilogue(c, k, acc):
                    # den > 0 always (the self-loop contributes), so no clamp;
                    # normalization rides the scalar engine (scale = 1/den).
                    rec = sb.tile([P, 1], f32, tag="rec")
                    nc.vector.reciprocal(rec[:], acc[:, HID:HID + 1])
                    hv = sb.tile([P, HID], f32, tag="hv")
                    nc.scalar.activation(hv[:], acc[:, 0:HID], Act.Identity,
                                         scale=rec[:])
                    if not is_last:
                        nc.vector.tensor_tensor(out=hv[:], in0=hv[:],
                                                in1=b1_t[:], op=Alu.add)
                        hv16 = sb.tile([P, HID], f16, tag="hv16")
                        nc.vector.tensor_scalar(out=hv16[:], in0=hv[:],
                                                scalar1=0.0, scalar2=None,
                                                op0=Alu.max)  # relu + cast
                        hvT_ps = ps.tile([HID, P], f16, tag="hvT")
                        nc.tensor.transpose(hvT_ps[:], hv16[:], id16_t[:])
                        hvT = sb.tile([HID, P], f16, tag="hvTs")
                        nc.vector.tensor_copy(hvT[:], hvT_ps[:])
                        t2_ps = ps.tile([P, HID + 2], f32, tag="epps")
                        nc.tensor.matmul(t2_ps[:], lhsT=hvT[:], rhs=w2_t[:],
                                         start=True, stop=True)
                        t2_sb = sb.tile([P, TW], f16, tag="t2sb")
                        ed2_sb = sb.tile([P, 1], f16, tag="t2ed")
                        node_transform(t2_ps, t2_sb, ed2_sb)
                        # writes issue from the scalar engine (the producer of
                        # t2_sb/ed2_sb) so they never block the sync engine's
                        # input-load stream for later blocks
                        nc.scalar.dma_start(
                            out_slice_ap[bass.ds(c * (CH * P) + k * P, P),
                                         0:TW],
                            t2_sb[:])
                        nc.scalar.dma_start(
                            out_eds_ap[c * CH + k:c * CH + k + 1, :]
                            .rearrange("a (b c) -> (a b) c", c=1),
                            ed2_sb[:])
                    else:
                        nc.vector.tensor_tensor(out=hv[:], in0=hv[:],
                                                in1=b2_t[:], op=Alu.add)
                        prhs = sb.tile([P, HM], f16, tag="prhs")
                        nc.vector.tensor_copy(prhs[:, 0:HID], hv[:])
                        nc.vector.tensor_copy(prhs[:, HID:HM], one16_t[:])
                        Gb = sb.tile([P, P], f16, tag="Gb")
                        nc.vector.tensor_tensor(
                            out=Gb[:],
                            in0=gid_t[:, bass.ds(c * CH + k, 1)]
                                .to_broadcast([P, P]),
                            in1=io32_t[:], op=Alu.is_equal)
                        pool_ps = ps.tile([P, HM], f32, tag="epps")
                        nc.tensor.matmul(pool_ps[:], lhsT=Gb[:], rhs=prhs[:],
                                         start=True, stop=True)
                        nc.vector.tensor_tensor(out=pooled[:], in0=pooled[:],
                                                in1=pool_ps[:], op=Alu.add)

                for c in range(nchunks):
                    # stream this chunk's gather indices
                    idxc = sb.tile([P, Q * CH * tqb * 8], i16, tag="idxc")
                    nc.sync.dma_start(
                        idxc[:], idx16_d[:, bass.ds(c * (Q * CH * tqb * 8),
                                                    Q * CH * tqb * 8)])
                    # all gathers for the chunk issued upfront, spread across
                    # the 4 SWDGE queues so descriptor generation overlaps
                    g_q = []
                    for q in range(Q):
                        gt = gq.tile([P, CH * tqb * ELEM], f16, tag="gq")
                        nc.gpsimd.dma_gather(
                            out_ap=gt[:].rearrange("p (n e) -> p n e", e=ELEM),
                            in_ap=table_ap[q * VQ:(q + 1) * VQ, :],
                            idxs_ap=idxc[:, bass.ds(q * (CH * tqb * 8),
                                                    CH * tqb * 8)],
                            num_idxs=NT, num_idxs_reg=NT, elem_size=ELEM,
                            single_packet=False, queue_num=q)
                        g_q.append(gt)

                    # software-pipeline: epilogue(k) emitted after prep(k+1)
                    pend = None
                    for k in range(CH):
                        acc = block_prep(c, k, g_q)
                        if pend is not None:
                            block_epilogue(c, pend[0], pend[1])
                        pend = (k, acc)
                    block_epilogue(c, pend[0], pend[1])

            gat_layer(table1, eds1, slice1, slice2, eds2, is_last=False)
            nc.gpsimd.collective_compute(
                "AllGather", Alu.bypass,
                replica_groups=[list(range(ncores))],
                ins=[slice2.opt()], outs=[table2[:, :]],
            )
            gat_layer(table2, eds2, slice2, None, None, is_last=True)

            # ---- AllReduce pooled sums ----
            nc.sync.dma_start(pool_in[:, :], pooled[:])
            nc.gpsimd.collective_compute(
                "AllReduce", Alu.add,
                replica_groups=[list(range(ncores))],
                ins=[pool_in.opt()], outs=[pool_out.opt()],
            )
            pl = sb.tile([P, HM], f32, tag="pl")
            nc.sync.dma_start(pl[:], pool_out[:, :])

            # mean = sum / max(count, 1)
            cnt = sb.tile([P, 1], f32, tag="cnt")
            nc.vector.tensor_scalar(out=cnt[:], in0=pl[:, HID:HID + 1],
                                    scalar1=1.0, scalar2=None, op0=Alu.max)
            crec = sb.tile([P, 1], f32, tag="crec")
            nc.vector.reciprocal(crec[:], cnt[:])
            mean = sb.tile([P, HID], f32, tag="mean")
            nc.vector.tensor_tensor(out=mean[:], in0=pl[:, 0:HID],
                                    in1=crec[:].to_broadcast([P, HID]), op=Alu.mult)

            # MLP: z = relu(mean @ lin_w + lin_b); logits = z @ cls_w + cls_b
            lw_t = cst.tile([HID, HID // 2], f32)
            nc.sync.dma_start(lw_t[:], lw_d[:, :])
            lb_t = cst.tile([HID // 2, 1], f32)
            nc.sync.dma_start(lb_t[:], lb_d[:, :])
            cw_t = cst.tile([HID // 2, C], f32)
            nc.sync.dma_start(cw_t[:], cw_d[:, :])
            cb_t = cst.tile([C, 1], f32)
            nc.sync.dma_start(cb_t[:], cb_d[:, :])

            meanT_ps = ps.tile([HID, P], f32, tag="pst")
            nc.tensor.transpose(meanT_ps[:], mean[:], id32_t[:])
            meanT = sb.tile([HID, P], f32, tag="meanTsb")
            nc.vector.tensor_copy(meanT[:], meanT_ps[:])
            zT_ps = ps.tile([HID // 2, P], f32, tag="pst")
            nc.tensor.matmul(zT_ps[:], lhsT=lw_t[:], rhs=meanT[:],
                             start=True, stop=True)
            zT = sb.tile([HID // 2, P], f32, tag="zTsb")
            nc.scalar.activation(zT[:], zT_ps[:], Act.Relu, bias=lb_t[:])
            lgT_ps = ps.tile([C, P], f32, tag="pst")
            nc.tensor.matmul(lgT_ps[:], lhsT=cw_t[:], rhs=zT[:],
                             start=True, stop=True)
            lgT = sb.tile([C, P], f32, tag="lgTsb")
            nc.scalar.activation(lgT[:], lgT_ps[:], Act.Identity, bias=cb_t[:])
            lg_ps = ps.tile([P, C], f32, tag="pst")
            nc.tensor.transpose(lg_ps[:], lgT[:], id32_t[:C, :C])
            lg = sb.tile([P, C], f32, tag="lgsb")
            nc.vector.tensor_copy(lg[:], lg_ps[:])

            # log_softmax over classes
            mx = sb.tile([P, 1], f32, tag="mx")
            nc.vector.tensor_reduce(mx[:], lg[:], axis=mybir.AxisListType.X,
                                    op=Alu.max)
            sh = sb.tile([P, C], f32, tag="sh")
            nc.vector.tensor_tensor(out=sh[:], in0=lg[:],
                                    in1=mx[:].to_broadcast([P, C]),
                                    op=Alu.subtract)
            exs = sb.tile([P, C], f32, tag="exs")
            se = sb.tile([P, 1], f32, tag="se")
            nc.scalar.activation(exs[:], sh[:], Act.Exp, accum_out=se[:])
            lse = sb.tile([P, 1], f32, tag="lse")
            nc.scalar.activation(lse[:], se[:], Act.Ln)
            res = sb.tile([P, C], f32, tag="res")
            nc.vector.tensor_tensor(out=res[:], in0=sh[:],
                                    in1=lse[:].to_broadcast([P, C]),
                                    op=Alu.subtract)
            nc.sync.dma_start(out_d[:, :], res[:])

    nc.compile()
    return nc


def run_gnn(inputs, ncores=8, trace=False):
    from concourse.bass_utils import run_bass_kernel_spmd

    x = np.asarray(inputs["x"], np.float32)
    edge_index = np.asarray(inputs["edge_index"])
    batch = np.asarray(inputs["batch"])
    W1 = np.asarray(inputs["W1"], np.float32)
    W2 = np.asarray(inputs["W2"], np.float32)
    hd = _build_host_data(
        x, edge_index, batch, W1,
        np.asarray(inputs["a_src1"], np.float32),
        np.asarray(inputs["a_dst1"], np.float32),
        W2,
        np.asarray(inputs["a_src2"], np.float32),
        np.asarray(inputs["a_dst2"], np.float32),
        ncores)

    N, F_IN = x.shape
    HID = W1.shape[1]
    G = 128  # number of graphs == P (pooling one-hot relies on this)
    C = np.asarray(inputs["cls_w"]).shape[1]

    nc = _build_program(ncores, hd["nblk"], hd["tqb"], hd["CH"], F_IN, HID, G, C)

    iota16 = np.tile(np.arange(P, dtype=np.float16)[None, :], (P, 1))
    iota32 = np.tile(np.arange(P, dtype=np.float32)[None, :], (P, 1))
    ident16 = np.eye(P, dtype=np.float16)
    ident32 = np.eye(P, dtype=np.float32)
    ones16 = np.ones((P, 1), np.float16)
    b1rep = np.tile(np.asarray(inputs["b1"], np.float32)[None, :], (P, 1))
    b2rep = np.tile(np.asarray(inputs["b2"], np.float32)[None, :], (P, 1))

    in_maps = []
    for c in range(ncores):
        in_maps.append({
            "xT": hd["xT_all"][c],
            "idx16": hd["idx16_all"][c],
            "sst": hd["SST_all"][c],
            "gid": hd["gid_all"][c],
            "w1aug": hd["W1aug"],
            "w2aug": hd["W2aug"],
            "b1rep": b1rep,
            "b2rep": b2rep,
            "lin_w": np.asarray(inputs["lin_w"], np.float32),
            "lin_b": np.asarray(inputs["lin_b"], np.float32)[:, None],
            "cls_w": np.asarray(inputs["cls_w"], np.float32),
            "cls_b": np.asarray(inputs["cls_b"], np.float32)[:, None],
            "iota32": iota32,
            "ident16": ident16,
            "ident32": ident32,
            "ones16": ones16,
        })

    res = run_bass_kernel_spmd(nc, in_maps, core_ids=list(range(ncores)),
                               trace=trace)
    out = res.results[0]["out"]
    return out, res


def kernel(**inputs):
    out, _ = run_gnn(inputs, ncores=8)
    return out.astype(np.float32)



# revision 38
# speedup vs baseline: 1.8490x; 1.8490x over previous
"""GAT (2-layer) + global mean pool + MLP + log_softmax on 8 Trainium2 cores.

v4 deltas over v3 (trace-driven):
  - S/ST one-hot tiles shipped as fp8 (halves the dominant DMA stream that
    was ~24ms of aggregate engine busy), converted to f16 on-chip with the
    convert split across the scalar (S) and vector (ST) engines.
  - Edge weight x applied to the gathered rhs rows ([P,QT,65], half the
    elements) instead of folding into the one-hot ([P,QT,128]).
  - AllGather outputs allocated in Shared DRAM scratchpad (fast HBM-HBM
    collective path; Local outputs warn + take a slow path).
  - Pool matmuls accumulate across a chunk in PSUM (one SBUF add per chunk).
  - Softmax denominator is strictly positive (self-loop), so the 1e-30
    clamp is dropped and normalization runs on the scalar engine
    (activation with per-partition scale=1/den).

Strategy (dst-sharded message passing, v3):
  - Nodes partitioned contiguously across 8 cores (12500 each). Each core
    aggregates messages for its own destination nodes.
  - Node tables are f16 rows [h@W (64) | 1 | exp(as) | exp(0.2*as)] padded
    to 128 elems (256B), built shard-wise, replicated via AllGather.  A
    per-core side table eds holds [exp(ad) | exp(0.2*ad)] per local node.
    Per-edge attention weight
      m = exp(leaky_relu(as+ad, 0.2)) == max(e^as*e^ad, e^.2as*e^.2ad)
    exactly, so the scalar engine only ever runs Exp on [128,1] columns
    per block - no per-edge activations, no activation-table thrash.
  - Gathers use gpsimd.dma_gather (one ~1us Q7 call amortized over
    thousands of 256B-row descriptors).  Its int16 index limit forces
    splitting the 100352-row table into 4 source "quarters"; edge cells
    are grouped (block, quarter) with per-(block,quarter) column padding.
    Self-loops are excluded from the gather entirely and added per block
    via an identity matmul on locally-loaded rows.
  - Per (block,quarter): one 3D-broadcast is_equal builds the one-hot
    tiles, one 3D mult scales rhs tiles by m (the table's constant-1
    column turns into the softmax denominator).  The PE accumulates
    acc[dst,0:65] = sum_e onehot^T @ (m*[h|1]) over edge tiles in PSUM.
  - Graph pooling = one-hot matmul per block summed in SBUF, AllReduce,
    and the tiny MLP + log_softmax run redundantly on every core.
"""

import sys

sys.path.insert(0, "/opt/trn_rl_repo")

import ml_dtypes
import numpy as np

F8 = ml_dtypes.float8_e4m3fn

P = 128
Q = 4          # source quarters (table split so indices fit int16)
ELEM = 128     # table row width in f16 elems (256B, dma_gather granularity)


def _wrap16(flat):
    """dma_gather index layout: index j at [j%16, j//16], replicated x8."""
    w = flat.reshape(-1, 16).T
    return np.tile(w, (8, 1)).astype(np.int16)


def _build_host_data(x, edge_index, batch, W1, a_src1, a_dst1, W2, a_src2, a_dst2,
                     ncores):
    """Pure-integer/graph preprocessing + augmented weights (host side)."""
    N, F_IN = x.shape
    HID = W1.shape[1]
    assert N % ncores == 0
    npc = N // ncores
    nblk = (npc + P - 1) // P
    npc_pad = nblk * P
    V = ncores * npc_pad
    assert V % Q == 0
    VQ = V // Q
    assert VQ <= 32768

    # self-loops are handled separately (identity matmul per block)
    src = edge_index[0].astype(np.int64)
    dst = edge_index[1].astype(np.int64)
    keep = src != dst
    src, dst = src[keep], dst[keep]

    src_pad = (src // npc) * npc_pad + (src % npc)
    core_of = dst // npc
    dst_loc = dst - core_of * npc
    blk_of = dst_loc // P
    dst_rel = dst_loc % P
    quart = src_pad // VQ

    counts = np.zeros((ncores, nblk, Q), np.int64)
    np.add.at(counts, (core_of, blk_of, quart), 1)
    tqb = int(max(1, np.ceil(counts.max() / P)))

    # chunk size: largest divisor of nblk with staging tiles <= ~9KB/partition
    CH = 1
    for ch in range(1, nblk + 1):
        if nblk % ch == 0 and ch * tqb * ELEM * 2 <= 10 * 1024:
            CH = ch
    nchunks = nblk // CH

    ncol_blk = Q * tqb
    ncols = nblk * ncol_blk

    idx16_all = np.zeros((ncores, P, ncols * 8), np.int16)
    # one-hot tiles (S) and their per-tile transposes (ST), uploaded as fp8:
    # S[e, col*128+c] = 1 iff edge at (e, col) targets dst_rel c
    S_all = np.zeros((ncores, P, ncols * P), F8)
    ST_all = np.zeros((ncores, P, ncols * P), F8)

    order = np.lexsort((dst_loc, quart, blk_of, core_of))
    co, bo, qo, ro, sp, dl = (core_of[order], blk_of[order], quart[order],
                              dst_rel[order], src_pad[order], dst_loc[order])
    key = (co * nblk + bo) * Q + qo
    start = np.searchsorted(key, np.arange(ncores * nblk * Q), side="left")
    pos = np.arange(len(key)) - start[key]
    t_of = pos // P
    p_of = pos % P
    chunk = bo // CH
    b_loc = bo % CH

    # block-major column id (block's Q*tqb columns contiguous)
    col = (bo * Q + qo) * tqb + t_of
    ro32 = ro.astype(np.int64)
    S_all[co, p_of, col * P + ro32] = 1.0
    ST_all[co, ro32, col * P + p_of] = 1.0

    # table-gather indices: call (chunk, q), flat j = (b_loc*tqb+t)*128 + p
    jf = (b_loc * tqb + t_of) * P + p_of
    call_col0 = chunk * (Q * CH * tqb * 8) + qo * (CH * tqb * 8)
    r16 = (jf % 16)
    c16 = jf // 16
    val = (sp - qo * VQ).astype(np.int16)
    for rep in range(8):
        idx16_all[co, r16 + 16 * rep, call_col0 + c16] = val

    gid_all = np.full((ncores, P, nblk), -1.0, np.float32)
    for c in range(ncores):
        ids = np.arange(npc) + c * npc
        g = batch[ids].astype(np.float32)
        gg = np.full(npc_pad, -1.0, np.float32)
        gg[:npc] = g
        gid_all[c] = gg.reshape(nblk, P).T

    xT_all = np.zeros((ncores, F_IN, npc_pad), np.float16)
    for c in range(ncores):
        xT_all[c, :, :npc] = x[c * npc:(c + 1) * npc].T.astype(np.float16)

    W1aug = np.concatenate([W1, (W1 @ a_src1)[:, None], (W1 @ a_dst1)[:, None]],
                           axis=1).astype(np.float16)
    W2aug = np.concatenate([W2, (W2 @ a_src2)[:, None], (W2 @ a_dst2)[:, None]],
                           axis=1).astype(np.float16)

    # interleave per block: [S_b | ST_b] so the device fetches both in one DMA
    ncol_blk2 = Q * tqb
    SST_all = np.concatenate(
        [S_all.reshape(ncores, P, nblk, ncol_blk2 * P),
         ST_all.reshape(ncores, P, nblk, ncol_blk2 * P)],
        axis=3).reshape(ncores, P, nblk * 2 * ncol_blk2 * P)

    return dict(npc=npc, nblk=nblk, npc_pad=npc_pad, tqb=tqb, ncols=ncols,
                CH=CH, nchunks=nchunks, VQ=VQ,
                SST_all=SST_all, idx16_all=idx16_all,
                gid_all=gid_all, xT_all=xT_all, W1aug=W1aug, W2aug=W2aug)


def _build_program(ncores, nblk, tqb, CH, F_IN, HID, G, C):
    import concourse.bass as bass
    import concourse.bacc as bacc
    import concourse.tile as tile
    from concourse import mybir

    TW = HID + 3          # used row prefix: [h | 1 | es | es2]
    HM = HID + 1          # matmul rhs width: [m*h | m]
    npc_pad = nblk * P
    V = ncores * npc_pad
    VQ = V // Q
    nchunks = nblk // CH
    ncols = nblk * Q * tqb
    NT = CH * tqb * P     # indices per table-gather call

    nc = bacc.Bacc("TRN2", target_bir_lowering=False, debug=False,
                   num_devices=ncores, num_swdge_queues=4)
    f32, f16 = mybir.dt.float32, mybir.dt.float16
    f8 = mybir.dt.float8e4
    i16 = mybir.dt.int16
    Alu = mybir.AluOpType
    Act = mybir.ActivationFunctionType

    ein = lambda n, s, d: nc.dram_tensor(n, s, d, kind="ExternalInput")
    xT_d = ein("xT", [F_IN, npc_pad], f16)
    idx16_d = ein("idx16", [P, ncols * 8], i16)
    # S and ST one-hot tiles interleaved per block: [S_b | ST_b], fp8
    sst_d = ein("sst", [P, ncols * P * 2], f8)
    gid_d = ein("gid", [P, nblk], f32)
    w1_d = ein("w1aug", [F_IN, HID + 2], f16)
    w2_d = ein("w2aug", [HID, HID + 2], f16)
    b1_d = ein("b1rep", [P, HID], f32)
    b2_d = ein("b2rep", [P, HID], f32)
    lw_d = ein("lin_w", [HID, HID // 2], f32)
    lb_d = ein("lin_b", [HID // 2, 1], f32)
    cw_d = ein("cls_w", [HID // 2, C], f32)
    cb_d = ein("cls_b", [C, 1], f32)
    io32_d = ein("iota32", [P, P], f32)
    id16_d = ein("ident16", [P, P], f16)
    id32_d = ein("ident32", [P, P], f32)
    one16_d = ein("ones16", [P, 1], f16)
    out_d = nc.dram_tensor("out", [G, C], f32, kind="ExternalOutput")

    with tile.TileContext(nc) as tc:
        with (
            tc.tile_pool(name="cst", bufs=1) as cst,
            tc.tile_pool(name="sb", bufs=6) as sb,
            tc.tile_pool(name="gq", bufs=10) as gq,
            tc.tile_pool(name="psA", bufs=2, space="PSUM") as psA,
            tc.tile_pool(name="ps", bufs=1, space="PSUM") as ps,
            tc.tile_pool(name="psacc", bufs=2, space="PSUM") as psacc,
            tc.tile_pool(name="dram", bufs=1, space="DRAM") as dram,
        ):
            # ---- constants ----
            gid_t = cst.tile([P, nblk], f32)
            nc.sync.dma_start(gid_t[:], gid_d[:, :])
            w1_t = cst.tile([F_IN, HID + 2], f16)
            nc.sync.dma_start(w1_t[:], w1_d[:, :])
            w2_t = cst.tile([HID, HID + 2], f16)
            nc.sync.dma_start(w2_t[:], w2_d[:, :])
            b1_t = cst.tile([P, HID], f32)
            nc.sync.dma_start(b1_t[:], b1_d[:, :])
            b2_t = cst.tile([P, HID], f32)
            nc.sync.dma_start(b2_t[:], b2_d[:, :])
            io32_t = cst.tile([P, P], f32)
            nc.sync.dma_start(io32_t[:], io32_d[:, :])
            id16_t = cst.tile([P, P], f16)
            nc.sync.dma_start(id16_t[:], id16_d[:, :])
            id32_t = cst.tile([P, P], f32)
            nc.sync.dma_start(id32_t[:], id32_d[:, :])
            one16_t = cst.tile([P, 1], f16)
            nc.sync.dma_start(one16_t[:], one16_d[:, :])

            # DRAM: AllGather staging slices + replicated tables.  Tables
            # live in the Shared scratchpad so the AllGather takes the fast
            # HBM-HBM path (Local outputs warn + bounce).
            slice1 = dram.tile([npc_pad, ELEM], f16)
            slice2 = dram.tile([npc_pad, ELEM], f16)
            table1 = nc.dram_tensor("table1_sh", [V, ELEM], f16,
                                    kind="Internal", addr_space="Shared")
            table2 = nc.dram_tensor("table2_sh", [V, ELEM], f16,
                                    kind="Internal", addr_space="Shared")
            pool_in = dram.tile([P, HM], f32)
            pool_out = dram.tile([P, HM], f32)

            # SBUF-resident per-core node tables (block-major columns):
            # [h|1|es|es2] rows plus the per-block exp(0.8*ad) columns.  All
            # per-block consumers (self-loop path, edps rhs, layer-2 build)
            # read these directly -- zero per-block DMA.  DRAM slices exist
            # only as AllGather staging, flushed per chunk.
            sl1_t = cst.tile([P, nblk * TW], f16)
            sl2_t = cst.tile([P, nblk * TW], f16)
            eds1_t = cst.tile([P, nblk], f16)
            eds2_t = cst.tile([P, nblk], f16)

            # pooled accumulator (SBUF, across all blocks of layer 2)
            pooled = cst.tile([P, HM], f32)
            nc.vector.memset(pooled[:], 0.0)

            def node_transform(src_ps, b0, nb, sl_t, eds_t):
                """src_ps[:, 0:nb*(HID+2)] = nb blocks of [h | as | ad] (f32,
                PSUM) -> sl_t cols b0..b0+nb = [h | 1 | e^as | e^.2as] (f16),
                eds_t cols b0..b0+nb = e^(0.8*ad) (f16)."""
                s3 = src_ps.rearrange("p (b c) -> p b c", b=nb)
                d3 = sl_t[:, bass.ds(b0 * TW, nb * TW)].rearrange(
                    "p (b c) -> p b c", b=nb)
                nc.scalar.activation(d3[:, :, 0:HID], s3[:, :, 0:HID],
                                     Act.Identity)
                nc.vector.tensor_copy(
                    d3[:, :, HID:HID + 1],
                    one16_t[:].unsqueeze(1).to_broadcast([P, nb, 1]))
                nc.scalar.activation(d3[:, :, HID + 1:HID + 2],
                                     s3[:, :, HID:HID + 1], Act.Exp)
                nc.scalar.activation(d3[:, :, HID + 2:HID + 3],
                                     s3[:, :, HID:HID + 1], Act.Exp, scale=0.2)
                nc.scalar.activation(
                    eds_t[:, bass.ds(b0, nb)].unsqueeze(2),
                    s3[:, :, HID + 1:HID + 2], Act.Exp, scale=0.8)

            def flush_slice(slice_dram, sl_t, c):
                """One DMA per chunk: SBUF node table -> DRAM AG staging."""
                nc.scalar.dma_start(
                    slice_dram[bass.ds(c * CH * P, CH * P), 0:TW]
                    .rearrange("(b p) c -> p b c", p=P),
                    sl_t[:, bass.ds(c * CH * TW, CH * TW)]
                    .rearrange("p (b c) -> p b c", b=CH))

            # ---- phase 0: table1 rows from x@W1aug (4-block batches) ----
            b = 0
            while b < nblk:
                nb = min(4, nblk - b)
                xb = sb.tile([F_IN, 4 * P], f16, tag="xb")
                nc.sync.dma_start(xb[:, 0:nb * P],
                                  xT_d[:, bass.ds(b * P, nb * P)])
                t1_ps = psA.tile([P, 4 * (HID + 2)], f32, tag="p0ps")
                for j in range(nb):
                    nc.tensor.matmul(
                        t1_ps[:, bass.ds(j * (HID + 2), HID + 2)],
                        lhsT=xb[:, bass.ds(j * P, P)], rhs=w1_t[:],
                        start=True, stop=True)
                node_transform(t1_ps[:, 0:nb * (HID + 2)], b, nb,
                               sl1_t, eds1_t)
                b += nb
            for c in range(nchunks):
                flush_slice(slice1, sl1_t, c)

            nc.gpsimd.collective_compute(
                "AllGather", Alu.bypass,
                replica_groups=[list(range(ncores))],
                ins=[slice1.opt()], outs=[table1[:, :]],
            )

            def gat_layer(table_ap, eds_t, sl_t, out_slice_dram, out_sl_t,
                          out_eds_t, is_last):
                def block_prep(c, k, g_q):
                    """Compute acc (PSUM) for block k of chunk c.

                    Attention weight per edge: m' = max(es_e*ed8_c, es2_e)
                    with ed8 = e^(0.8*ad).  The exact weight is
                    e^(0.2*ad_c) * m', but that factor is constant per dst
                    row and cancels in the softmax numerator/denominator."""
                    # self-loop row for this block: read straight from the
                    # SBUF-resident node table (no DMA)
                    bb = c * CH + k
                    g_self = sl_t[:, bass.ds(bb * TW, TW)]
                    ed8col = eds_t[:, bb:bb + 1]
                    u_s = sb.tile([P, 1], f16, tag="uws")
                    nc.vector.tensor_tensor(out=u_s[:],
                                            in0=g_self[:, HID + 1:HID + 2],
                                            in1=ed8col, op=Alu.mult)
                    m_s = sb.tile([P, 1], f16, tag="ms")
                    nc.vector.tensor_tensor(out=m_s[:], in0=u_s[:],
                                            in1=g_self[:, HID + 2:HID + 3],
                                            op=Alu.max)
                    rhs_s = sb.tile([P, HM], f16, tag="rhss")
                    nc.vector.tensor_tensor(out=rhs_s[:],
                                            in0=g_self[:, 0:HM],
                                            in1=m_s[:].to_broadcast([P, HM]),
                                            op=Alu.mult)

                    acc = psacc.tile([P, HM], f32, tag="acc")
                    nc.tensor.matmul(acc[:], lhsT=id16_t[:], rhs=rhs_s[:],
                                     start=True, stop=False)

                    QT = Q * tqb
                    # stream this block's one-hot tiles (host-precomputed fp8,
                    # S and ST in one DMA); the PE consumes fp8 lhsT directly
                    sst_up = sb.tile([P, 2 * QT * P], f8, tag="SST")
                    nc.sync.dma_start(
                        sst_up[:],
                        sst_d[:, bass.ds((c * CH + k) * 2 * QT * P,
                                         2 * QT * P)])
                    S8 = sst_up[:, 0:QT * P]
                    ST8 = sst_up[:, QT * P:2 * QT * P]

                    # per-edge ed8 of the dst via transposed one-hot matmuls
                    edps = ps.tile([P, QT], f32, tag="edR")
                    for j in range(QT):
                        nc.tensor.matmul(edps[:, j:j + 1],
                                         lhsT=ST8[:, bass.ds(j * P, P)],
                                         rhs=ed8col, start=True, stop=True)

                    # per-edge [es|es2] pairs for all 4 quarters into one tile
                    esb = sb.tile([P, QT * 2], f16, tag="uw")
                    for q in range(Q):
                        g3 = g_q[q][:].rearrange("p (n e) -> p n e", e=ELEM)[
                            :, k * tqb:(k + 1) * tqb, :]
                        nc.vector.tensor_copy(
                            esb[:, bass.ds(q * tqb * 2, tqb * 2)]
                                .rearrange("p (t c) -> p t c", t=tqb),
                            g3[:, :, HID + 1:HID + 3])
                    es3 = esb[:].rearrange("p (t c) -> p t c", t=QT)
                    # x_e = max(es_e * ed8_dst(e), es2_e)  [P, QT]
                    x = sb.tile([P, QT], f16, tag="m")
                    nc.vector.tensor_tensor(
                        out=x[:].unsqueeze(2), in0=es3[:, :, 0:1],
                        in1=edps[:].unsqueeze(2), op=Alu.mult)
                    nc.vector.tensor_tensor(
                        out=x[:].unsqueeze(2), in0=x[:].unsqueeze(2),
                        in1=es3[:, :, 1:2], op=Alu.max)
                    # scale the gathered rhs rows by x ([h|1] -> x*[h|1], so
                    # acc col 64 accumulates the softmax denominator); cheaper
                    # than folding x into the 128-wide one-hot
                    rhs_s = sb.tile([P, QT * HM], f16, tag="rhsS")
                    for q in range(Q):
                        g3 = g_q[q][:].rearrange("p (n e) -> p n e", e=ELEM)[
                            :, k * tqb:(k + 1) * tqb, :]
                        nc.vector.tensor_tensor(
                            out=rhs_s[:, bass.ds(q * tqb * HM, tqb * HM)]
                                .rearrange("p (t c) -> p t c", t=tqb),
                            in0=g3[:, :, 0:HM],
                            in1=x[:, bass.ds(q * tqb, tqb)].unsqueeze(2)
                                .to_broadcast([P, tqb, HM]),
                            op=Alu.mult)
                    for j in range(QT):
                        nc.tensor.matmul(
                            acc[:],
                            lhsT=S8[:, bass.ds(j * P, P)],
                            rhs=rhs_s[:, bass.ds(j * HM, HM)],
                            start=False,
                            stop=(j == QT - 1))
                    return acc

                def block_epilogue(c, k, acc):
                    # den > 0 always (the self-loop contributes), so no clamp;
                    # normalization rides the scalar engine (scale = 1/den).
                    rec = sb.tile([P, 1], f32, tag="rec")
                    nc.vector.reciprocal(rec[:], acc[:, HID:HID + 1])
                    hv = sb.tile([P, HID], f32, tag="hv")
                    nc.scalar.activation(hv[:], acc[:, 0:HID], Act.Identity,
                                         scale=rec[:])
                    if not is_last:
                        nc.vector.tensor_tensor(out=hv[:], in0=hv[:],
                                                in1=b1_t[:], op=Alu.add)
                        hv16 = sb.tile([P, HID], f16, tag="hv16")
                        nc.vector.tensor_scalar(out=hv16[:], in0=hv[:],
                                                scalar1=0.0, scalar2=None,
                                                op0=Alu.max)  # relu + cast
                        hvT_ps = ps.tile([HID, P], f16, tag="hvT")
                        nc.tensor.transpose(hvT_ps[:], hv16[:], id16_t[:])
                        hvT = sb.tile([HID, P], f16, tag="hvTs")
                        nc.vector.tensor_copy(hvT[:], hvT_ps[:])
                        t2_ps = ps.tile([P, HID + 2], f32, tag="epps")
                        nc.tensor.matmul(t2_ps[:], lhsT=hvT[:], rhs=w2_t[:],
                                         start=True, stop=True)
                        # write layer-2 node rows into the SBUF table; DRAM
                        # staging is flushed once per chunk
                        node_transform(t2_ps[:], c * CH + k, 1,
                                       out_sl_t, out_eds_t)
                    else:
                        nc.vector.tensor_tensor(out=hv[:], in0=hv[:],
                                                in1=b2_t[:], op=Alu.add)
                        prhs = sb.tile([P, HM], f16, tag="prhs")
                        nc.vector.tensor_copy(prhs[:, 0:HID], hv[:])
                        nc.vector.tensor_copy(prhs[:, HID:HM], one16_t[:])
                        Gb = sb.tile([P, P], f16, tag="Gb")
                        nc.vector.tensor_tensor(
                            out=Gb[:],
                            in0=gid_t[:, bass.ds(c * CH + k, 1)]
                                .to_broadcast([P, P]),
                            in1=io32_t[:], op=Alu.is_equal)
                        pool_ps = ps.tile([P, HM], f32, tag="epps")
                        nc.tensor.matmul(pool_ps[:], lhsT=Gb[:], rhs=prhs[:],
                                         start=True, stop=True)
                        nc.vector.tensor_tensor(out=pooled[:], in0=pooled[:],
                                                in1=pool_ps[:], op=Alu.add)

                for c in range(nchunks):
                    # stream this chunk's gather indices
                    idxc = sb.tile([P, Q * CH * tqb * 8], i16, tag="idxc")
                    nc.sync.dma_start(
                        idxc[:], idx16_d[:, bass.ds(c * (Q * CH * tqb * 8),
                                                    Q * CH * tqb * 8)])
                    # all gathers for the chunk issued upfront, spread across
                    # the 4 SWDGE queues so descriptor generation overlaps
                    g_q = []
                    for q in range(Q):
                        gt = gq.tile([P, CH * tqb * ELEM], f16, tag="gq")
                        nc.gpsimd.dma_gather(
                            out_ap=gt[:].rearrange("p (n e) -> p n e", e=ELEM),
                            in_ap=table_ap[q * VQ:(q + 1) * VQ, :],
                            idxs_ap=idxc[:, bass.ds(q * (CH * tqb * 8),
                                                    CH * tqb * 8)],
                            num_idxs=NT, num_idxs_reg=NT, elem_size=ELEM,
                            single_packet=False, queue_num=q)
                        g_q.append(gt)

                    # software-pipeline: epilogue(k) emitted after prep(k+1)
                    pend = None
                    for k in range(CH):
                        acc = block_prep(c, k, g_q)
                        if pend is not None:
                            block_epilogue(c, pend[0], pend[1])
                        pend = (k, acc)
                    block_epilogue(c, pend[0], pend[1])
                    if not is_last:
                        flush_slice(out_slice_dram, out_sl_t, c)

            gat_layer(table1, eds1_t, sl1_t, slice2, sl2_t, eds2_t,
                      is_last=False)
            nc.gpsimd.collective_compute(
                "AllGather", Alu.bypass,
                replica_groups=[list(range(ncores))],
                ins=[slice2.opt()], outs=[table2[:, :]],
            )
            gat_layer(table2, eds2_t, sl2_t, None, None, None, is_last=True)

            # ---- AllReduce pooled sums ----
            nc.sync.dma_start(pool_in[:, :], pooled[:])
            nc.gpsimd.collective_compute(
                "AllReduce", Alu.add,
                replica_groups=[list(range(ncores))],
                ins=[pool_in.opt()], outs=[pool_out.opt()],
            )
            pl = sb.tile([P, HM], f32, tag="pl")
            nc.sync.dma_start(pl[:], pool_out[:, :])

            # mean = sum / max(count, 1)
            cnt = sb.tile([P, 1], f32, tag="cnt")
            nc.vector.tensor_scalar(out=cnt[:], in0=pl[:, HID:HID + 1],
                                    scalar1=1.0, scalar2=None, op0=Alu.max)
            crec = sb.tile([P, 1], f32, tag="crec")
            nc.vector.reciprocal(crec[:], cnt[:])
            mean = sb.tile([P, HID], f32, tag="mean")
            nc.vector.tensor_tensor(out=mean[:], in0=pl[:, 0:HID],
                                    in1=crec[:].to_broadcast([P, HID]), op=Alu.mult)

            # MLP: z = relu(mean @ lin_w + lin_b); logits = z @ cls_w + cls_b
            lw_t = cst.tile([HID, HID // 2], f32)
            nc.sync.dma_start(lw_t[:], lw_d[:, :])
            lb_t = cst.tile([HID // 2, 1], f32)
            nc.sync.dma_start(lb_t[:], lb_d[:, :])
            cw_t = cst.tile([HID // 2, C], f32)
            nc.sync.dma_start(cw_t[:], cw_d[:, :])
            cb_t = cst.tile([C, 1], f32)
            nc.sync.dma_start(cb_t[:], cb_d[:, :])

            meanT_ps = ps.tile([HID, P], f32, tag="pst")
            nc.tensor.transpose(meanT_ps[:], mean[:], id32_t[:])
            meanT = sb.tile([HID, P], f32, tag="meanTsb")
            nc.vector.tensor_copy(meanT[:], meanT_ps[:])
            zT_ps = ps.tile([HID // 2, P], f32, tag="pst")
            nc.tensor.matmul(zT_ps[:], lhsT=lw_t[:], rhs=meanT[:],
                             start=True, stop=True)
            zT = sb.tile([HID // 2, P], f32, tag="zTsb")
            nc.scalar.activation(zT[:], zT_ps[:], Act.Relu, bias=lb_t[:])
            lgT_ps = ps.tile([C, P], f32, tag="pst")
            nc.tensor.matmul(lgT_ps[:], lhsT=cw_t[:], rhs=zT[:],
                             start=True, stop=True)
            lgT = sb.tile([C, P], f32, tag="lgTsb")
            nc.scalar.activation(lgT[:], lgT_ps[:], Act.Identity, bias=cb_t[:])
            lg_ps = ps.tile([P, C], f32, tag="pst")
            nc.tensor.transpose(lg_ps[:], lgT[:], id32_t[:C, :C])
            lg = sb.tile([P, C], f32, tag="lgsb")
            nc.vector.tensor_copy(lg[:], lg_ps[:])

            # log_softmax over classes
            mx = sb.tile([P, 1], f32, tag="mx")
            nc.vector.tensor_reduce(mx[:], lg[:], axis=mybir.AxisListType.X,
                                    op=Alu.max)
            sh = sb.tile([P, C], f32, tag="sh")
            nc.vector.tensor_tensor(out=sh[:], in0=lg[:],
                                    in1=mx[:].to_broadcast([P, C]),
                                    op=Alu.subtract)
            exs = sb.tile([P, C], f32, tag="exs")
            se = sb.tile([P, 1], f32, tag="se")
            nc.scalar.activation(exs[:], sh[:], Act.Exp, accum_out=se[:])
            lse = sb.tile([P, 1], f32, tag="lse")
            nc.scalar.activation(lse[:], se[:], Act.Ln)
            res = sb.tile([P, C], f32, tag="res")
            nc.vector.tensor_tensor(out=res[:], in0=sh[:],
                                    in1=lse[:].to_broadcast([P, C]),
                                    op=Alu.subtract)
            nc.sync.dma_start(out_d[:, :], res[:])

    nc.compile()
    return nc


def run_gnn(inputs, ncores=8, trace=False):
    from concourse.bass_utils import run_bass_kernel_spmd

    x = np.asarray(inputs["x"], np.float32)
    edge_index = np.asarray(inputs["edge_index"])
    batch = np.asarray(inputs["batch"])
    W1 = np.asarray(inputs["W1"], np.float32)
    W2 = np.asarray(inputs["W2"], np.float32)
    hd = _build_host_data(
        x, edge_index, batch, W1,
        np.asarray(inputs["a_src1"], np.float32),
        np.asarray(inputs["a_dst1"], np.float32),
        W2,
        np.asarray(inputs["a_src2"], np.float32),
        np.asarray(inputs["a_dst2"], np.float32),
        ncores)

    N, F_IN = x.shape
    HID = W1.shape[1]
    G = 128  # number of graphs == P (pooling one-hot relies on this)
    C = np.asarray(inputs["cls_w"]).shape[1]

    nc = _build_program(ncores, hd["nblk"], hd["tqb"], hd["CH"], F_IN, HID, G, C)

    iota16 = np.tile(np.arange(P, dtype=np.float16)[None, :], (P, 1))
    iota32 = np.tile(np.arange(P, dtype=np.float32)[None, :], (P, 1))
    ident16 = np.eye(P, dtype=np.float16)
    ident32 = np.eye(P, dtype=np.float32)
    ones16 = np.ones((P, 1), np.float16)
    b1rep = np.tile(np.asarray(inputs["b1"], np.float32)[None, :], (P, 1))
    b2rep = np.tile(np.asarray(inputs["b2"], np.float32)[None, :], (P, 1))

    in_maps = []
    for c in range(ncores):
        in_maps.append({
            "xT": hd["xT_all"][c],
            "idx16": hd["idx16_all"][c],
            "sst": hd["SST_all"][c],
            "gid": hd["gid_all"][c],
            "w1aug": hd["W1aug"],
            "w2aug": hd["W2aug"],
            "b1rep": b1rep,
            "b2rep": b2rep,
            "lin_w": np.asarray(inputs["lin_w"], np.float32),
            "lin_b": np.asarray(inputs["lin_b"], np.float32)[:, None],
            "cls_w": np.asarray(inputs["cls_w"], np.float32),
            "cls_b": np.asarray(inputs["cls_b"], np.float32)[:, None],
            "iota32": iota32,
            "ident16": ident16,
            "ident32": ident32,
            "ones16": ones16,
        })

    res = run_bass_kernel_spmd(nc, in_maps, core_ids=list(range(ncores)),
                               trace=trace)
    out = res.results[0]["out"]
    return out, res


def kernel(**inputs):
    out, _ = run_gnn(inputs, ncores=8)
    return out.astype(np.float32)

